# revision 72
# baseline (speedup 1.0000x reference)
"""Single-head causal attention (B=8, T=2048, C=768, H=64) on 8 TRN2 NeuronCores.

Sharding: data-parallel over the batch dim — one batch element per core.

Per-core algorithm (v6 — fp8 K projection, fused Q|V, exp-paced pipeline):
  - host feeds xT [C, T] bf16, wqv = [Wq | Wv] packed [P, NCH*P] bf16,
    wk8 = fp8(32*Wk) packed [P, NCH*H].
  - xT streamed via the 3 HWDGE queues (sync/scalar/gpsimd): first the
    [0:512) cols of all 6 C-chunks, then [512:1024), then the back half.
  - x8: xT cast bf16 -> fp8e4 on VectorE (feeds the K projection).
  - QV projection (bf16): fused [Wq | Wv] stationary; rows 0:64 = q^T,
    rows 64:128 = v^T. K projection via fp8 DoubleRow (2 C-chunks per
    matmul) with k scaled by 32 (folded into the exp scale); k lands at
    base partition 0.
  - v^T PE-transposed (base-64 identity) into v tiles [128, 65] + ones col.
  - attention in S^T layout: S^T(j, i-range) = kT_j.T @ qT; exp on ScalarE
    (PSUM -> SBUF bf16, scale = C**-0.5/32); causal diagonal block masked
    in place by GpSimd affine_select. AV: out^T [65, i] += [v_j | 1].T @ P^T.
  - ONE PSUM ring (2 x [128,1024] f32) carries warmup matmuls, QV groups and
    S^T tiles; attention pairs start as soon as QV0/K0 land, and the
    remaining projection groups are woven between early pairs. The kernel is
    paced by the ScalarE exp chain (~21us at ~118G elem/s).
  - HAM discipline: the PE clock gate defaults to 1.2 GHz and only sustained
    near-full duty holds 2.4 GHz; one mostly-idle 3.4us window collapses the
    rest of the kernel to half clock. Warmup matmuls bridge the (HBM-
    contended) input DMA, and dummy LDWEIGHTS "fillers" — data-dependent on
    the previous pair's probabilities so the topological scheduler cannot
    hoist them — absorb the PE's per-pair wait on the exp chain.
  - DMA: the HWDGE queues round-robin packets across ALL queued descriptors
    and sync/scalar queues outrank gpsimd's, so transfers are issued in
    dependency-gated waves (probe DMAs reading freshly-landed data block an
    engine's later issues) with weights/group0 first.
  - finalize per 512-col half as soon as its last AV lands: PSUM -> SBUF
    bf16 copy, PE-transpose [65,128] -> [128,65] (bf16), reciprocal of the
    denominator row, tensor_scalar multiply, DMA out [T, H] fp32.
"""

import ml_dtypes
import numpy as np

import concourse.bass as bass
import concourse.tile as tile
from concourse import bacc, mybir
from concourse.bass import ds, ts
from concourse.masks import make_identity

B, T, C, H = 8, 2048, 768, 64
P = 128
NCH = C // P          # 6 contraction chunks for QKV
GW = 1024             # attention output column-group width
NG = T // GW          # 2 groups
NT = T // P           # 16 t-chunks
JPG = GW // P         # 8 j-chunks per group
WSC = 32.0            # host-side scale on Wk before fp8 cast
SCALE = float(C) ** -0.5 / WSC
N_WARMUP = 7          # PE warmup matmuls: un-throttle HAM while x streams in

F32 = mybir.dt.float32
BF16 = mybir.dt.bfloat16
FP8 = mybir.dt.float8e4
EXP = mybir.ActivationFunctionType.Exp
DR = mybir.MatmulPerfMode.DoubleRow


def _emit(tc: tile.TileContext, ctx, xgs, wqv, wk8, out):
    nc = tc.nc

    consts = ctx.enter_context(tc.tile_pool(name="consts", bufs=1))
    xpool = ctx.enter_context(tc.tile_pool(name="x", bufs=1))
    qpool = ctx.enter_context(tc.tile_pool(name="qkv", bufs=1))

    ident = consts.tile([H, H], BF16)
    make_identity(nc, ident[:])
    ident_hi = consts.tile([P, H], BF16)
    nc.gpsimd.memset(ident_hi[:], 0.0)
    ident65 = consts.tile([H + 1, H + 1], BF16)
    make_identity(nc, ident65[:])



    qvT = qpool.tile([P, T], BF16)   # rows 0:64 q^T, rows 64:128 v^T
    kT = qpool.tile([H, T], BF16)    # k^T * 32
    v_sb = qpool.tile([P, NT, H + 1], BF16)
    dum = qpool.tile([P, 512], BF16)
    nc.vector.memset(dum[:], 0.0)
    probe = consts.tile([2, 512], BF16)

    # weights host-packed to [P, 8*dim] (rows padded to a power-of-2 byte
    # count — non-pow2 runs transfer pathologically slowly on the DMA engines)
    w_qv = consts.tile([P, 8, P], BF16)
    nc.sync.dma_start(w_qv[:], wqv.rearrange("p (o m) -> p o m", o=8))
    w_k8 = consts.tile([P, 8, H], FP8)
    nc.sync.dma_start(w_k8[:], wk8.rearrange("p (o m) -> p o m", o=8))
    # identity copy to partitions 64:128 (for v^T transposes)
    nc.sync.dma_start(ident_hi[H:P, :], ident[:])

    # x host-packed per 512-col group as [P, NCH*512] (6KB contiguous rows).
    # The DMA queues round-robin across ALL queued descriptors, so later
    # groups steal bandwidth from earlier ones; issue in two waves — wave A
    # (groups 0-1) over all three queues at kernel start, wave B (groups 2-3)
    # over sync+gpsimd, held back by a probe DMA that reads qvT group 0 (the
    # issuing engine blocks until the QV0 copy lands, by which time wave A
    # has drained).
    xg_sb = [xpool.tile([P, NCH, 512], BF16, name=f"xg{g}") for g in range(4)]
    x8g = [xpool.tile([P, NCH, 512], FP8, name=f"x8g{g}") for g in range(4)]
    xr = [xgs[g].rearrange("p (o m) -> p o m", o=NCH) for g in range(4)]
    # Queue priority is sync ~ scalar >> gpsimd (gpsimd's queue is starved
    # while the others pump). So: groups 0-1 ride sync+scalar, with probe
    # DMAs (reads of freshly-landed group-0 data) holding group 1 back so it
    # doesn't round-robin with group 0; groups 2-3 ride the gpsimd queue,
    # naturally starved until the front queues drain — exactly when they're
    # needed — leaving the gpsimd ENGINE free for the diagonal masks.
    # keep the sync queue shallow until the weights land (packet-level fair
    # sharing means every queued descriptor slows every other one down)
    nc.sync.dma_start(probe[0:1, 256:384], w_qv[0:1, 7, 0:128])
    qA = [(nc.sync, ds(0, 2)), (nc.scalar, ds(2, 2)), (nc.scalar, ds(4, 2))]
    for q, cl in qA:
        q.dma_start(xg_sb[0][:, cl, :], xr[0][:, cl, :])
    nc.sync.dma_start(probe[0:1, 0:128], xg_sb[0][0:1, 1, 0:128])
    nc.scalar.dma_start(probe[0:1, 128:256], xg_sb[0][0:1, 5, 0:128])
    for q, cl in qA:
        q.dma_start(xg_sb[1][:, cl, :], xr[1][:, cl, :])
    nc.sync.dma_start(probe[1:2, 0:128], xg_sb[1][0:1, 1, 0:128])
    nc.sync.dma_start(xg_sb[2][:], xr[2][:])
    nc.gpsimd.dma_start(probe[1:2, 128:256], xg_sb[1][0:1, 3, 0:128])
    nc.gpsimd.dma_start(xg_sb[3][:], xr[3][:])

    # ---- pools: one unified PSUM budget (<= 16KB/partition) ----
    sp = ctx.enter_context(tc.tile_pool(name="spsum", bufs=2, space="PSUM"))
    kp = ctx.enter_context(tc.tile_pool(name="kpsum", bufs=1, space="PSUM"))
    op = ctx.enter_context(tc.tile_pool(name="opsum", bufs=1, space="PSUM"))
    fp = ctx.enter_context(tc.tile_pool(name="fpsum", bufs=1, space="PSUM"))
    pb = ctx.enter_context(tc.tile_pool(name="probs", bufs=6))
    fin = ctx.enter_context(tc.tile_pool(name="fin", bufs=3))

    def cast_g(g):
        nc.vector.tensor_copy(x8g[g][:], xg_sb[g][:])

    for w in range(N_WARMUP):
        dps = sp.tile([P, GW], F32, tag="s", name=f"warm_{w}")
        nc.tensor.matmul(dps[:, 0:512], dum[:, 0:P], dum[:], start=True, stop=True)

    cast_g(0)

    def emit_qv(g):
        ps = sp.tile([P, GW], F32, tag="s", name=f"qv_{g}")
        for c in range(NCH):
            nc.tensor.matmul(
                ps[:, 0:512],
                w_qv[:, c, :],
                xg_sb[g][:, c, :],
                start=(c == 0),
                stop=(c == NCH - 1),
            )
        nc.vector.tensor_copy(qvT[:, ts(g, 512)], ps[:, 0:512])

    def emit_k(g):
        kps = kp.tile([H, 512], F32, tag="k", name=f"k_{g}")
        for h2 in range(2):
            for p in range(NCH // 2):
                nc.tensor.matmul(
                    kps[:, ts(h2, 256)],
                    w_k8[:, 2 * p : 2 * p + 2, :],
                    x8g[g][:, 2 * p : 2 * p + 2, ts(h2, 256)],
                    start=(p == 0),
                    stop=(p == NCH // 2 - 1),
                    perf_mode=DR,
                )
        nc.vector.tensor_copy(kT[:, ts(g, 512)], kps[:])

    def emit_transposes(g):
        for t in range(4 * g, 4 * g + 4):
            pt = fp.tile([P, H + 1], BF16, tag="ft", name=f"vt_{t}")
            nc.tensor.transpose(pt[:, 0:H], qvT[H:P, ts(t, P)], ident_hi[H:P, :])
            nc.vector.tensor_copy(v_sb[:, t, 0:H], pt[:, 0:H])

    emit_qv(0)
    emit_k(0)
    emit_qv(1)
    cast_g(1)
    emit_k(1)

    # ---- attention (remaining projection groups woven between pairs) ----
    def emit_probs(g, jj):
        istart = max(g * GW, jj * P)
        n = (g + 1) * GW - istart
        sps = sp.tile([P, GW], F32, tag="s", name=f"s_{g}_{jj}")
        for h in range(0, n, 512):
            nh = min(512, n - h)
            nc.tensor.matmul(
                sps[:, h : h + nh],
                kT[:, ts(jj, P)],
                qvT[0:H, ds(istart + h, nh)],
                start=True,
                stop=True,
            )
        prb = pb.tile([P, GW], BF16, tag="p")
        nc.scalar.activation(prb[:, :n], sps[:, :n], EXP, scale=SCALE)
        if jj >= JPG * g:
            # leading 128 cols are the diagonal block: zero j > i in place
            nc.gpsimd.affine_select(
                out=prb[:, :P],
                in_=prb[:, :P],
                compare_op=mybir.AluOpType.is_ge,
                fill=0.0,
                base=0,
                pattern=[[1, P]],
                channel_multiplier=-1,
            )
        return prb

    def emit_finalize_half(g, hh, ops, tail=False):
        ot = fin.tile([H + 1, 512], BF16, tag="ot", name=f"ot_{g}_{hh}")
        nc.vector.tensor_copy(ot[:], ops[:, ts(hh, 512)])
        for t8 in range(4):
            tt = g * (GW // P) + hh * 4 + t8
            if tail and t8 % 2 == 1:
                # the sps ring is idle by now; alternating pools lets the
                # PE-transpose / VectorE ping-pong pipeline instead of
                # serializing on the single fpsum buffer
                ptf = sp.tile([P, GW], F32, tag="s", name=f"ft_{tt}")
                ptf = ptf.bitcast(BF16)[:, 0 : H + 1]
            else:
                ptf = fp.tile([P, H + 1], BF16, tag="ft", name=f"ft_{tt}")
            nc.tensor.transpose(ptf[:], ot[:, ts(t8, P)], ident65[:])
            rch = fin.tile([P, 1], F32, tag="rch", name=f"rch_{tt}")
            nc.vector.reciprocal(rch[:], ptf[:, H : H + 1])
            o_nat = fin.tile([P, H], F32, tag="onat", name=f"onat_{tt}")
            nc.vector.tensor_scalar_mul(o_nat[:], ptf[:, 0:H], rch[:])
            nc.sync.dma_start(out[ts(tt, P), :], o_nat[:])
            if tail:
                for _ in range(6):
                    nc.tensor.ldweights(ot[:, ts(t8, P)])

    # remaining phase-1 work, keyed by the pair index it is emitted before.
    # Groups 2-3 land at ~18-20us, and nothing needs them before pair 8
    # (S of group 1 reads qvT cols 1024:2048) — weave them late enough that
    # the group-0 exp chain never waits on them.
    def weave(idx):
        if idx == 2:
            emit_transposes(0)
            nc.vector.memset(v_sb[:, 0:4, H : H + 1], 1.0)
        elif idx == 3:
            emit_transposes(1)
            nc.vector.memset(v_sb[:, 4:8, H : H + 1], 1.0)
        elif idx == 4:
            cast_g(2)
            emit_qv(2)
        elif idx == 5:
            emit_k(2)
            cast_g(3)
        elif idx == 6:
            emit_qv(3)
            emit_transposes(2)
            nc.vector.memset(v_sb[:, 8:12, H : H + 1], 1.0)
        elif idx == 7:
            emit_k(3)
            emit_transposes(3)
            nc.vector.memset(v_sb[:, 12:16, H : H + 1], 1.0)

    def filler(prb, k):
        # dummy weight loads holding the PE HAM duty above its un-throttle
        # point: the exp-paced attention sits near the threshold, and a single
        # cold 3.4us window collapses the whole phase to 1.2 GHz. LDWEIGHTS
        # touches no PSUM and every real matmul reloads its own stationary.
        # Reading the pair's prb tile pins them to the pair's schedule slot
        # (the tile framework schedules topologically, not by emission order).
        for _ in range(k):
            nc.tensor.ldweights(prb[:, 0:P])

    pairs = [(g, jj) for g in range(NG) for jj in range(JPG * g + JPG)]
    # per (g, half): ordered list of writing j-chunks, for start/stop flags
    seg_writers = {}
    for g, jj in pairs:
        for hh in range(2):
            if jj * P - g * GW < (hh + 1) * 512:
                seg_writers.setdefault((g, hh), []).append(jj)

    ops_by_g = {}
    pending_halves = []
    LOOKAHEAD = 2
    prb_queue = []
    for i in range(LOOKAHEAD):
        weave(i)
        prb_queue.append(emit_probs(*pairs[i]))
    last_prb = prb_queue[0]
    for idx, (g, jj) in enumerate(pairs):
        prb = prb_queue.pop(0)
        if idx + LOOKAHEAD < len(pairs):
            weave(idx + LOOKAHEAD)
            prb_queue.append(emit_probs(*pairs[idx + LOOKAHEAD]))
        # fillers run right after S(p+2)'s matmuls, absorbing the PE's wait
        # for exp(p); they read prb(p-1), which is guaranteed complete, so
        # the scheduler can't hoist them ahead of the attention stream
        filler(last_prb, 3)
        last_prb = prb

        if g not in ops_by_g:
            ops_by_g[g] = op.tile([H + 1, GW], F32, tag="o", name=f"ops_{g}")
        ops = ops_by_g[g]
        istart = max(g * GW, jj * P)
        n = (g + 1) * GW - istart
        ioff = istart - g * GW
        # split at the ops tile's absolute 512-col PSUM bank boundaries
        seg = ioff
        while seg < ioff + n:
            seg_end = min(ioff + n, (seg // 512 + 1) * 512)
            half = seg // 512
            ws = seg_writers[(g, half)]
            nc.tensor.matmul(
                ops[:, seg:seg_end],
                v_sb[:, jj, :],
                prb[:, seg - ioff : seg_end - ioff],
                start=(jj == ws[0]),
                stop=(jj == ws[-1]),
            )
            seg = seg_end

        # each 512-wide half finalizes as soon as its last AV lands, delayed
        # by one pair so the copy/transposes overlap the following matmuls
        while pending_halves:
            emit_finalize_half(*pending_halves.pop(0))
        for hh in range(2):
            if seg_writers[(g, hh)][-1] == jj:
                pending_halves.append((g, hh, ops))
    while pending_halves:
        emit_finalize_half(*pending_halves.pop(0), tail=True)


def build():
    from contextlib import ExitStack

    nc = bacc.Bacc("TRN2", target_bir_lowering=False, debug=False, num_devices=B)
    xgs = [
        nc.dram_tensor(f"xg{g}", [P, NCH * 512], BF16, kind="ExternalInput").ap()
        for g in range(4)
    ]
    wqv = nc.dram_tensor("wqv", [P, 8 * P], BF16, kind="ExternalInput").ap()
    wk8 = nc.dram_tensor("wk8", [P, 8 * H], FP8, kind="ExternalInput").ap()
    out = nc.dram_tensor("o", [T, H], F32, kind="ExternalOutput").ap()
    with tile.TileContext(nc) as tc, ExitStack() as ctx:
        _emit(tc, ctx, xgs, wqv, wk8, out)
    nc.compile()
    return nc


_NC = None


def _get_nc():
    global _NC
    if _NC is None:
        _NC = build()
    return _NC


def _pack_w(w):
    # [C, M] -> [P, 8*M]: partition-major chunk layout, contiguous rows,
    # padded from NCH=6 to 8 chunks so each row is a power-of-2 byte count
    M = w.shape[1]
    p = np.zeros((P, 8, M), dtype=w.dtype)
    p[:, :NCH, :] = w.reshape(NCH, P, M).transpose(1, 0, 2)
    return np.ascontiguousarray(p.reshape(P, 8 * M))


def make_in_maps(x, Wk, Wq, Wv):
    bf = ml_dtypes.bfloat16
    f8 = ml_dtypes.float8_e4m3
    wqv = _pack_w(np.concatenate([Wq, Wv], axis=1)).astype(bf)
    wk8 = _pack_w(np.asarray(Wk) * WSC).astype(f8)
    maps = []
    for b in range(B):
        xTb = np.asarray(x[b]).T.astype(bf)  # [C, T]
        m = {"wqv": wqv, "wk8": wk8}
        for g in range(4):
            xg = xTb[:, g * 512 : (g + 1) * 512]
            m[f"xg{g}"] = np.ascontiguousarray(
                xg.reshape(NCH, P, 512).transpose(1, 0, 2).reshape(P, NCH * 512)
            )
        maps.append(m)
    return maps


def kernel(x, Wk, Wq, Wv):
    from concourse.bass_utils import run_bass_kernel_spmd

    nc = _get_nc()
    in_maps = make_in_maps(x, Wk, Wq, Wv)
    r = run_bass_kernel_spmd(nc, in_maps, core_ids=list(range(B)))
    out = np.stack([r.results[b]["o"] for b in range(B)])
    return np.ascontiguousarray(out, dtype=np.float32)
